# revision 1
# baseline (speedup 1.0000x reference)
"""Expert-parallel MoE kernel for Trainium2 (8 NeuronCores, Bass/Tile).

Sharding: expert dim E=256 split 32-per-core across 8 cores; router is
evaluated on the host (128x256 — negligible) and each core receives its
local experts' weights plus the per-token combine weights for those
experts. Each core computes the combine-weighted partial output of its
32 experts; the host sums the 8 partials. No device collectives needed.

Weights are host-packed per expert PAIR into one contiguous block
[128 partitions x 16384 fp32] holding w1 (pre-tiled [i, k, h]) then w2
([i, k, d]); each pair streams as 4 perfectly-linear 2MB DMAs on the SP
HWDGE ring (b1 rides the gpsimd SWDGE ring so the weight stream never
stalls), measured ~97% of single-core HBM line rate.
Matmuls use float32r (fp32 bits, relaxed PE mode,
1 cyc/row) with x-transposed as the stationary operand so the streamed
weights are the moving operand. Per expert: h matmuls -> erf-GELU on
ScalarE (bias added via ones-row matmul) -> fold top-k combine weight in
with a per-partition VectorE scale -> PE transpose -> second matmul
accumulating all experts into one PSUM bank; + one K=32 matmul for the
b2 term; single output DMA.
"""

import numpy as np

B, T, DIM = 2, 64, 512
E, H, K = 256, 1024, 42
N = B * T                     # 128 tokens
N_CORES = 8
EPC = E // N_CORES            # 32 experts per core
GP = EPC // 2                 # 16 expert pairs per core

# fp32 bits, relaxed-precision PE mode (1 cyc/row at N>=256 vs 4 for exact
# fp32). Flip to "float32" if accuracy demands.
MM_DTYPE = "float32r"

W1B = 4 * H                   # fp32 elements of one expert's w1 per partition
W2B = 8 * DIM                 # fp32 elements of one expert's w2 per partition
PAIRW = 2 * (W1B + W2B)       # 16384 elements per partition per pair

_prog_cache = {}


def _build_program(mm_dtype_name, act="Gelu", n_pairs=GP, repeat=1,
                   wsplit=4, rings=("sync",), group=2, wbufs=2,
                   b1eng="gpsimd"):
    from contextlib import ExitStack

    import concourse.bacc as bacc
    import concourse.mybir as mybir
    import concourse.tile as tile

    f32 = mybir.dt.float32
    # Matmul operands are declared in the matmul dtype end-to-end (the BIR
    # verifier requires fp32r consumers to see fp32r producers). For
    # float32r the bits are plain fp32 on the host side.
    mdt = getattr(mybir.dt, mm_dtype_name)
    GELU = getattr(mybir.ActivationFunctionType, act)

    KD = DIM // 128          # 4 contraction slices for x @ w1
    KH = H // 128            # 8 contraction slices for h @ w2
    NSEG = H // 512          # 2 PSUM halves for h

    nc = bacc.Bacc("TRN2", target_bir_lowering=False, debug=False,
                   num_devices=N_CORES)

    xT_d = nc.dram_tensor("xT", [DIM, N], mdt, kind="ExternalInput")
    n_grp = EPC // group
    grpw = group * (W1B + W2B)
    wpk_d = nc.dram_tensor("wpk", [n_grp, 128, grpw], mdt, kind="ExternalInput")
    b1_d = nc.dram_tensor("b1s", [EPC, H], mdt, kind="ExternalInput")
    b2_d = nc.dram_tensor("b2s", [EPC, DIM], mdt, kind="ExternalInput")
    cc_d = nc.dram_tensor("combc", [N, EPC], f32, kind="ExternalInput")
    ct_d = nc.dram_tensor("combT", [EPC, N], mdt, kind="ExternalInput")
    id_d = nc.dram_tensor("ident", [128, 128], f32, kind="ExternalInput")
    ones_d = nc.dram_tensor("ones", [1, N], mdt, kind="ExternalInput")
    out_d = nc.dram_tensor("out", [N, DIM], f32, kind="ExternalOutput")

    with tile.TileContext(nc) as tc, ExitStack() as ctx:
        const = ctx.enter_context(tc.tile_pool(name="const", bufs=1))
        wp = ctx.enter_context(tc.tile_pool(name="wp", bufs=wbufs))
        b1p = ctx.enter_context(tc.tile_pool(name="b1p", bufs=2))
        hgp = ctx.enter_context(tc.tile_pool(name="hgp", bufs=2))
        hTsp = ctx.enter_context(tc.tile_pool(name="hTsp", bufs=2))
        outp = ctx.enter_context(tc.tile_pool(name="outp", bufs=1))
        hps = ctx.enter_context(tc.tile_pool(name="hps", bufs=2, space="PSUM"))
        hTps = ctx.enter_context(tc.tile_pool(name="hTps", bufs=1, space="PSUM"))
        yps = ctx.enter_context(tc.tile_pool(name="yps", bufs=1, space="PSUM"))

        xT_sb = const.tile([128, KD * N], mdt)
        nc.sync.dma_start(
            xT_sb[:].rearrange("p (k t) -> p k t", k=KD),
            xT_d[:, :].rearrange("(k p) t -> p k t", p=128),
        )
        id_sb = const.tile([128, 128], f32)
        nc.sync.dma_start(id_sb[:], id_d[:, :])
        cc_sb = const.tile([N, EPC], f32)
        nc.sync.dma_start(cc_sb[:], cc_d[:, :])
        ct_sb = const.tile([EPC, N], mdt)
        nc.sync.dma_start(ct_sb[:], ct_d[:, :])
        b2_sb = const.tile([EPC, DIM], mdt)
        nc.sync.dma_start(b2_sb[:], b2_d[:, :])
        ones_sb = const.tile([1, N], mdt)
        nc.sync.dma_start(ones_sb[:], ones_d[:, :])

        y_ps = yps.tile([N, DIM], f32)

        def emit_experts():
            for g in range(n_grp):
                w_t = wp.tile([128, grpw], mdt)
                csz = grpw // wsplit
                for ci in range(wsplit):
                    eng = getattr(nc, rings[ci % len(rings)])
                    eng.dma_start(w_t[:, ci * csz : (ci + 1) * csz],
                                  wpk_d[g][:, ci * csz : (ci + 1) * csz])
                b1_t = b1p.tile([1, group * H], mdt)
                getattr(nc, b1eng).dma_start(
                    b1_t[:].rearrange("o (i h) -> o i h", i=group),
                    b1_d[group * g : group * (g + 1), :].rearrange(
                        "(o i) h -> o i h", o=1),
                )
                for i in range(group):
                    e = group * g + i
                    h_ps = hps.tile([N, H], f32)
                    for s in range(NSEG):
                        seg = slice(s * 512, (s + 1) * 512)
                        for k in range(KD):
                            nc.tensor.matmul(
                                h_ps[:, seg],
                                lhsT=xT_sb[:, k * N : (k + 1) * N],
                                rhs=w_t[:, i * W1B + k * H + s * 512 :
                                        i * W1B + k * H + s * 512 + 512],
                                start=(k == 0), stop=False,
                            )
                        nc.tensor.matmul(
                            h_ps[:, seg],
                            lhsT=ones_sb[:],
                            rhs=b1_t[:, i * H + s * 512 : i * H + (s + 1) * 512],
                            start=False, stop=True,
                        )

                    hg = hgp.tile([N, H], f32)
                    nc.scalar.activation(hg[:], h_ps[:], GELU)
                    nc.vector.tensor_scalar_mul(hg[:], hg[:], cc_sb[:, e : e + 1])

                    hT_ps = hTps.tile([128, H], f32)
                    for j in range(KH):
                        nc.tensor.transpose(
                            hT_ps[:, j * 128 : (j + 1) * 128],
                            hg[:, j * 128 : (j + 1) * 128],
                            id_sb[:],
                        )
                    hT_sb = hTsp.tile([128, H], mdt)
                    nc.vector.tensor_copy(hT_sb[:], hT_ps[:])

                    w2off = group * W1B + i * W2B
                    for j in range(KH):
                        nc.tensor.matmul(
                            y_ps[:],
                            lhsT=hT_sb[:, j * 128 : (j + 1) * 128],
                            rhs=w_t[:, w2off + j * DIM : w2off + (j + 1) * DIM],
                            start=(e == 0 and j == 0), stop=False,
                        )

        if repeat > 1:
            # timing-only variant: re-run the whole expert sweep on-device
            # to amortize host/tunnel dispatch overhead. hint_engines arms
            # back-edge branch prefetch for the >256-inst PE/sync bodies so
            # the loop edge costs ~0.3us instead of a ~4us IRAM refetch.
            hint = (mybir.EngineType.PE, mybir.EngineType.SP)
            with tc.For_i(0, repeat, 1, hint_engines=hint):
                emit_experts()
        else:
            emit_experts()

        nc.tensor.matmul(
            y_ps[:], lhsT=ct_sb[:], rhs=b2_sb[:],
            start=False, stop=True,
        )
        o_sb = outp.tile([N, DIM], f32)
        nc.vector.tensor_copy(o_sb[:], y_ps[:])
        nc.sync.dma_start(out_d[:, :], o_sb[:])

    nc.compile()
    return nc


def get_program(mm_dtype_name=MM_DTYPE, act="Gelu", n_pairs=GP, repeat=1,
                wsplit=4, rings=("sync",), group=2, wbufs=2,
                b1eng="gpsimd"):
    key = (mm_dtype_name, act, n_pairs, repeat, wsplit, tuple(rings), group,
           wbufs, b1eng)
    if key not in _prog_cache:
        _prog_cache[key] = _build_program(mm_dtype_name, act, n_pairs, repeat,
                                          wsplit, rings, group, wbufs, b1eng)
    return _prog_cache[key]


def _softmax(v, axis=-1):
    m = np.max(v, axis=axis, keepdims=True)
    ex = np.exp(v - m)
    return ex / np.sum(ex, axis=axis, keepdims=True)


def host_routing(x, router_w, router_b):
    """Replicates the reference routing in fp32 numpy: softmax over all
    experts, take top-K probs, renormalize those with another softmax."""
    xt = np.asarray(x, np.float32).reshape(N, DIM)
    logits = xt @ np.asarray(router_w, np.float32) + np.asarray(router_b, np.float32)
    probs = _softmax(logits, axis=-1)
    idx = np.argpartition(probs, E - K, axis=-1)[:, E - K:]          # top-K set
    vals = np.take_along_axis(probs, idx, axis=-1)
    w = _softmax(vals, axis=-1)
    comb = np.zeros((N, E), np.float32)
    np.put_along_axis(comb, idx, w.astype(np.float32), axis=-1)
    return comb


def pack_weights(w1c, w2c, group=2):
    """[32,512,1024] + [32,1024,512] -> [32/group, 128, group*12288]:
    per expert group, per partition, [w1(i,k,h) | w2(i,k,d)] contiguous."""
    ng = EPC // group
    a = (w1c.reshape(ng, group, KD_, 128, H).transpose(0, 3, 1, 2, 4)
         .reshape(ng, 128, group * W1B))
    b = (w2c.reshape(ng, group, KH_, 128, DIM).transpose(0, 3, 1, 2, 4)
         .reshape(ng, 128, group * W2B))
    return np.ascontiguousarray(np.concatenate([a, b], axis=2))


KD_ = DIM // 128
KH_ = H // 128


def make_in_maps(x, w1, b1, w2, b2, router_w, router_b, group=2):
    x = np.ascontiguousarray(np.asarray(x, np.float32))
    w1 = np.asarray(w1, np.float32)
    b1 = np.asarray(b1, np.float32)
    w2 = np.asarray(w2, np.float32)
    b2 = np.asarray(b2, np.float32)
    comb = host_routing(x, router_w, router_b)
    xT = np.ascontiguousarray(x.reshape(N, DIM).T)
    ident = np.eye(128, dtype=np.float32)
    in_maps = []
    for c in range(N_CORES):
        sl = slice(c * EPC, (c + 1) * EPC)
        cl = np.ascontiguousarray(comb[:, sl])
        in_maps.append({
            "xT": xT,
            "wpk": pack_weights(w1[sl], w2[sl], group),
            "b1s": np.ascontiguousarray(b1[sl]),
            "b2s": np.ascontiguousarray(b2[sl]),
            "combc": cl,
            "combT": np.ascontiguousarray(cl.T),
            "ident": ident,
            "ones": np.ones((1, N), np.float32),
        })
    return in_maps


def kernel(x, w1, b1, w2, b2, router_w, router_b):
    from concourse.bass_utils import run_bass_kernel_spmd

    nc = get_program()
    in_maps = make_in_maps(x, w1, b1, w2, b2, router_w, router_b)
    res = run_bass_kernel_spmd(nc, in_maps, list(range(N_CORES)))
    out = np.zeros((N, DIM), np.float32)
    for r in res.results:
        out += r["out"]
    return out.reshape(B, T, DIM).astype(np.float32)



# revision 8
# speedup vs baseline: 1.8602x; 1.8602x over previous
"""Expert-parallel MoE kernel for Trainium2 (8 NeuronCores, Bass/Tile).

Sharding: expert dim E=256 split 32-per-core across 8 cores; router is
evaluated on the host (128x256 — negligible) and each core receives its
local experts' weights plus the per-token combine weights for those
experts. Each core computes the combine-weighted partial output of its
32 experts; the host sums the 8 partials. No device collectives needed.

Weights are host-packed per expert PAIR into one contiguous block
[128 partitions x 16384 fp32] holding w1 (pre-tiled [i, k, h]) then w2
([i, k, d]); each pair streams as 4 perfectly-linear 2MB DMAs on the SP
HWDGE ring (b1 rides the gpsimd SWDGE ring so the weight stream never
stalls), measured ~97% of single-core HBM line rate.
Matmuls use float32r (fp32 bits, relaxed PE mode,
1 cyc/row) with x-transposed as the stationary operand so the streamed
weights are the moving operand. Per expert: h matmuls -> erf-GELU on
ScalarE (bias added via ones-row matmul) -> fold top-k combine weight in
with a per-partition VectorE scale -> PE transpose -> second matmul
accumulating all experts into one PSUM bank; + one K=32 matmul for the
b2 term; single output DMA.
"""

import numpy as np

B, T, DIM = 2, 64, 512
E, H, K = 256, 1024, 42
N = B * T                     # 128 tokens
N_CORES = 8
EPC = E // N_CORES            # 32 experts per core
GP = EPC // 2                 # 16 expert pairs per core

# bf16 datapath: halves the HBM weight stream vs fp32 (the kernel is
# memory-bound), PE matmuls run 1 cyc/row. Measured end-to-end rel err
# ~3e-3 vs the fp32 reference (budget 2e-2). Flip back to "float32r" if
# accuracy demands.
MM_DTYPE = "bfloat16"

W1B = 4 * H                   # fp32 elements of one expert's w1 per partition
W2B = 8 * DIM                 # fp32 elements of one expert's w2 per partition
PAIRW = 2 * (W1B + W2B)       # 16384 elements per partition per pair

_prog_cache = {}


def _build_program(mm_dtype_name, act="Gelu", n_pairs=GP, repeat=1,
                   wsplit=4, rings=("sync",), group=2, wbufs=2,
                   b1eng="gpsimd"):
    from contextlib import ExitStack

    import concourse.bacc as bacc
    import concourse.mybir as mybir
    import concourse.tile as tile

    f32 = mybir.dt.float32
    # Matmul operands are declared in the matmul dtype end-to-end (the BIR
    # verifier requires matching producer/consumer dtypes). For bfloat16
    # the host passes ml_dtypes.bfloat16 arrays.
    mdt = getattr(mybir.dt, mm_dtype_name)
    GELU = getattr(mybir.ActivationFunctionType, act)

    KD = DIM // 128          # 4 contraction slices for x @ w1
    KH = H // 128            # 8 contraction slices for h @ w2
    NSEG = H // 512          # 2 PSUM halves for h

    nc = bacc.Bacc("TRN2", target_bir_lowering=False, debug=False,
                   num_devices=N_CORES)

    xT_d = nc.dram_tensor("xT", [DIM, N], mdt, kind="ExternalInput")
    n_grp = EPC // group
    grpw = group * (W1B + W2B)
    wpk_d = nc.dram_tensor("wpk", [n_grp, 128, grpw], mdt, kind="ExternalInput")
    b1_d = nc.dram_tensor("b1s", [EPC, H], mdt, kind="ExternalInput")
    b2_d = nc.dram_tensor("b2s", [EPC, DIM], mdt, kind="ExternalInput")
    cc_d = nc.dram_tensor("combc", [N, EPC], f32, kind="ExternalInput")
    ct_d = nc.dram_tensor("combT", [EPC, N], mdt, kind="ExternalInput")
    id_d = nc.dram_tensor("ident", [128, 128], mdt, kind="ExternalInput")
    ones_d = nc.dram_tensor("ones", [1, N], mdt, kind="ExternalInput")
    out_d = nc.dram_tensor("out", [N, DIM], f32, kind="ExternalOutput")

    with tile.TileContext(nc) as tc, ExitStack() as ctx:
        const = ctx.enter_context(tc.tile_pool(name="const", bufs=1))
        wp = ctx.enter_context(tc.tile_pool(name="wp", bufs=wbufs))
        b1p = ctx.enter_context(tc.tile_pool(name="b1p", bufs=2))
        hgp = ctx.enter_context(tc.tile_pool(name="hgp", bufs=2))
        hqp = ctx.enter_context(tc.tile_pool(name="hqp", bufs=2))
        hTsp = ctx.enter_context(tc.tile_pool(name="hTsp", bufs=2))
        outp = ctx.enter_context(tc.tile_pool(name="outp", bufs=1))
        hps = ctx.enter_context(tc.tile_pool(name="hps", bufs=2, space="PSUM"))
        hTps = ctx.enter_context(tc.tile_pool(name="hTps", bufs=1, space="PSUM"))
        yps = ctx.enter_context(tc.tile_pool(name="yps", bufs=1, space="PSUM"))

        xT_sb = const.tile([128, KD * N], mdt)
        nc.sync.dma_start(
            xT_sb[:].rearrange("p (k t) -> p k t", k=KD),
            xT_d[:, :].rearrange("(k p) t -> p k t", p=128),
        )
        id_sb = const.tile([128, 128], mdt)
        nc.sync.dma_start(id_sb[:], id_d[:, :])
        cc_sb = const.tile([N, EPC], f32)
        nc.sync.dma_start(cc_sb[:], cc_d[:, :])
        ct_sb = const.tile([EPC, N], mdt)
        nc.sync.dma_start(ct_sb[:], ct_d[:, :])
        b2_sb = const.tile([EPC, DIM], mdt)
        nc.sync.dma_start(b2_sb[:], b2_d[:, :])
        ones_sb = const.tile([1, N], mdt)
        nc.sync.dma_start(ones_sb[:], ones_d[:, :])

        y_ps = yps.tile([N, DIM], f32)

        def emit_experts():
            for g in range(n_grp):
                w_t = wp.tile([128, grpw], mdt)
                csz = grpw // wsplit
                for ci in range(wsplit):
                    eng = getattr(nc, rings[ci % len(rings)])
                    eng.dma_start(w_t[:, ci * csz : (ci + 1) * csz],
                                  wpk_d[g][:, ci * csz : (ci + 1) * csz])
                b1_t = b1p.tile([1, group * H], mdt)
                getattr(nc, b1eng).dma_start(
                    b1_t[:].rearrange("o (i h) -> o i h", i=group),
                    b1_d[group * g : group * (g + 1), :].rearrange(
                        "(o i) h -> o i h", o=1),
                )
                for i in range(group):
                    e = group * g + i
                    h_ps = hps.tile([N, H], f32)
                    for s in range(NSEG):
                        seg = slice(s * 512, (s + 1) * 512)
                        for k in range(KD):
                            nc.tensor.matmul(
                                h_ps[:, seg],
                                lhsT=xT_sb[:, k * N : (k + 1) * N],
                                rhs=w_t[:, i * W1B + k * H + s * 512 :
                                        i * W1B + k * H + s * 512 + 512],
                                start=(k == 0), stop=False,
                            )
                        nc.tensor.matmul(
                            h_ps[:, seg],
                            lhsT=ones_sb[:],
                            rhs=b1_t[:, i * H + s * 512 : i * H + (s + 1) * 512],
                            start=False, stop=True,
                        )

                    hg = hgp.tile([N, H], f32)
                    nc.scalar.activation(hg[:], h_ps[:], GELU)
                    # fold the top-k combine weight in per-partition; the DVE
                    # write also downcasts fp32 -> mdt for the transpose
                    hq = hqp.tile([N, H], mdt)
                    nc.vector.tensor_scalar_mul(hq[:], hg[:], cc_sb[:, e : e + 1])

                    hT_ps = hTps.tile([128, H], mdt)
                    for j in range(KH):
                        nc.tensor.transpose(
                            hT_ps[:, j * 128 : (j + 1) * 128],
                            hq[:, j * 128 : (j + 1) * 128],
                            id_sb[:],
                        )
                    hT_sb = hTsp.tile([128, H], mdt)
                    nc.vector.tensor_copy(hT_sb[:], hT_ps[:])

                    w2off = group * W1B + i * W2B
                    for j in range(KH):
                        nc.tensor.matmul(
                            y_ps[:],
                            lhsT=hT_sb[:, j * 128 : (j + 1) * 128],
                            rhs=w_t[:, w2off + j * DIM : w2off + (j + 1) * DIM],
                            start=(e == 0 and j == 0), stop=False,
                        )

        if repeat > 1:
            # timing-only variant: re-run the whole expert sweep on-device
            # to amortize host/tunnel dispatch overhead. hint_engines arms
            # back-edge branch prefetch for the >256-inst PE/sync bodies so
            # the loop edge costs ~0.3us instead of a ~4us IRAM refetch.
            hint = (mybir.EngineType.PE, mybir.EngineType.SP)
            with tc.For_i(0, repeat, 1, hint_engines=hint):
                emit_experts()
        else:
            emit_experts()

        nc.tensor.matmul(
            y_ps[:], lhsT=ct_sb[:], rhs=b2_sb[:],
            start=False, stop=True,
        )
        o_sb = outp.tile([N, DIM], f32)
        nc.vector.tensor_copy(o_sb[:], y_ps[:])
        nc.sync.dma_start(out_d[:, :], o_sb[:])

    nc.compile()
    return nc


def get_program(mm_dtype_name=MM_DTYPE, act="Gelu", n_pairs=GP, repeat=1,
                wsplit=4, rings=("sync",), group=2, wbufs=2,
                b1eng="gpsimd"):
    key = (mm_dtype_name, act, n_pairs, repeat, wsplit, tuple(rings), group,
           wbufs, b1eng)
    if key not in _prog_cache:
        _prog_cache[key] = _build_program(mm_dtype_name, act, n_pairs, repeat,
                                          wsplit, rings, group, wbufs, b1eng)
    return _prog_cache[key]


def _softmax(v, axis=-1):
    m = np.max(v, axis=axis, keepdims=True)
    ex = np.exp(v - m)
    return ex / np.sum(ex, axis=axis, keepdims=True)


def host_routing(x, router_w, router_b):
    """Replicates the reference routing in fp32 numpy: softmax over all
    experts, take top-K probs, renormalize those with another softmax."""
    xt = np.asarray(x, np.float32).reshape(N, DIM)
    logits = xt @ np.asarray(router_w, np.float32) + np.asarray(router_b, np.float32)
    probs = _softmax(logits, axis=-1)
    idx = np.argpartition(probs, E - K, axis=-1)[:, E - K:]          # top-K set
    vals = np.take_along_axis(probs, idx, axis=-1)
    w = _softmax(vals, axis=-1)
    comb = np.zeros((N, E), np.float32)
    np.put_along_axis(comb, idx, w.astype(np.float32), axis=-1)
    return comb


def pack_weights(w1c, w2c, group=2):
    """[32,512,1024] + [32,1024,512] -> [32/group, 128, group*12288]:
    per expert group, per partition, [w1(i,k,h) | w2(i,k,d)] contiguous."""
    ng = EPC // group
    a = (w1c.reshape(ng, group, KD_, 128, H).transpose(0, 3, 1, 2, 4)
         .reshape(ng, 128, group * W1B))
    b = (w2c.reshape(ng, group, KH_, 128, DIM).transpose(0, 3, 1, 2, 4)
         .reshape(ng, 128, group * W2B))
    return np.ascontiguousarray(np.concatenate([a, b], axis=2))


KD_ = DIM // 128
KH_ = H // 128


def make_in_maps(x, w1, b1, w2, b2, router_w, router_b, group=2):
    import ml_dtypes

    hdt = np.float32 if MM_DTYPE in ("float32", "float32r") else getattr(
        ml_dtypes, MM_DTYPE)
    x = np.ascontiguousarray(np.asarray(x, np.float32))
    w1 = np.asarray(w1, hdt)
    b1 = np.asarray(b1, hdt)
    w2 = np.asarray(w2, hdt)
    b2 = np.asarray(b2, hdt)
    comb = host_routing(x, router_w, router_b)
    xT = np.ascontiguousarray(x.reshape(N, DIM).T.astype(hdt))
    ident = np.eye(128, dtype=hdt)
    in_maps = []
    for c in range(N_CORES):
        sl = slice(c * EPC, (c + 1) * EPC)
        cl = np.ascontiguousarray(comb[:, sl])
        in_maps.append({
            "xT": xT,
            "wpk": pack_weights(w1[sl], w2[sl], group),
            "b1s": np.ascontiguousarray(b1[sl]),
            "b2s": np.ascontiguousarray(b2[sl]),
            "combc": cl,
            "combT": np.ascontiguousarray(cl.T.astype(hdt)),
            "ident": ident,
            "ones": np.ones((1, N), hdt),
        })
    return in_maps


def kernel(x, w1, b1, w2, b2, router_w, router_b):
    from concourse.bass_utils import run_bass_kernel_spmd

    nc = get_program()
    in_maps = make_in_maps(x, w1, b1, w2, b2, router_w, router_b)
    res = run_bass_kernel_spmd(nc, in_maps, list(range(N_CORES)))
    out = np.zeros((N, DIM), np.float32)
    for r in res.results:
        out += r["out"]
    return out.reshape(B, T, DIM).astype(np.float32)



# revision 15
# speedup vs baseline: 1.9766x; 1.0626x over previous
"""Expert-parallel MoE kernel for Trainium2 (8 NeuronCores, Bass/Tile).

Sharding: expert dim E=256 split 32-per-core across 8 cores; router is
evaluated on the host (128x256 — negligible) and each core receives its
local experts' weights plus the per-token combine weights for those
experts. Each core computes the combine-weighted partial output of its
32 experts; the host sums the 8 partials. No device collectives needed.

Weights are host-packed per expert PAIR into one contiguous block
[128 partitions x 16384 fp32] holding w1 (pre-tiled [i, k, h]) then w2
([i, k, d]); each pair streams as 4 perfectly-linear 2MB DMAs on the SP
HWDGE ring (b1 rides the gpsimd SWDGE ring so the weight stream never
stalls), measured ~97% of single-core HBM line rate.
Matmuls use float32r (fp32 bits, relaxed PE mode,
1 cyc/row) with x-transposed as the stationary operand so the streamed
weights are the moving operand. Per expert: h matmuls -> erf-GELU on
ScalarE (bias added via ones-row matmul) -> fold top-k combine weight in
with a per-partition VectorE scale -> PE transpose -> second matmul
accumulating all experts into one PSUM bank; + one K=32 matmul for the
b2 term; single output DMA.
"""

import numpy as np

B, T, DIM = 2, 64, 512
E, H, K = 256, 1024, 42
N = B * T                     # 128 tokens
N_CORES = 8
EPC = E // N_CORES            # 32 experts per core
GP = EPC // 2                 # 16 expert pairs per core

# bf16 datapath: halves the HBM weight stream vs fp32 (the kernel is
# memory-bound), PE matmuls run 1 cyc/row. Measured end-to-end rel err
# ~3e-3 vs the fp32 reference (budget 2e-2). Flip back to "float32r" if
# accuracy demands.
MM_DTYPE = "bfloat16"
# w2 streams in fp8 e3m4 (4 mantissa bits), scaled by W2_SCALE so the
# values sit in e3m4's normal range (max 15.5); the 1/W2_SCALE
# compensation is folded into the combine weights on the host, so the
# device math is unchanged. PE mixed-dtype matmul (bf16 stationary x
# fp8e3 moving) verified exact on HW. Emulated end-to-end rel err
# 1.2e-2 (budget 2e-2). Set to None to keep w2 in MM_DTYPE.
W2_DTYPE = "float8e3"
W2_SCALE = 256.0

W1B = 4 * H                   # fp32 elements of one expert's w1 per partition
W2B = 8 * DIM                 # fp32 elements of one expert's w2 per partition
PAIRW = 2 * (W1B + W2B)       # 16384 elements per partition per pair

_prog_cache = {}


def _build_program(mm_dtype_name, act="Gelu", n_pairs=GP, repeat=1,
                   wsplit=4, rings=("sync",), group=2, wbufs=2,
                   b1eng="gpsimd", w2_dtype_name=None, w2split=2):
    from contextlib import ExitStack

    import concourse.bacc as bacc
    import concourse.mybir as mybir
    import concourse.tile as tile

    f32 = mybir.dt.float32
    # Matmul operands are declared in the matmul dtype end-to-end (the BIR
    # verifier requires matching producer/consumer dtypes). For bfloat16
    # the host passes ml_dtypes.bfloat16 arrays.
    mdt = getattr(mybir.dt, mm_dtype_name)
    w2dt = getattr(mybir.dt, w2_dtype_name) if w2_dtype_name else mdt
    GELU = getattr(mybir.ActivationFunctionType, act)

    KD = DIM // 128          # 4 contraction slices for x @ w1
    KH = H // 128            # 8 contraction slices for h @ w2
    NSEG = H // 512          # 2 PSUM halves for h

    nc = bacc.Bacc("TRN2", target_bir_lowering=False, debug=False,
                   num_devices=N_CORES)

    xT_d = nc.dram_tensor("xT", [DIM, N], mdt, kind="ExternalInput")
    n_grp = EPC // group
    g1w = group * W1B
    g2w = group * W2B
    w1_d = nc.dram_tensor("wpk1", [n_grp, 128, g1w], mdt, kind="ExternalInput")
    w2_d = nc.dram_tensor("wpk2", [n_grp, 128, g2w], w2dt, kind="ExternalInput")
    b1_d = nc.dram_tensor("b1s", [EPC, H], mdt, kind="ExternalInput")
    b2_d = nc.dram_tensor("b2s", [EPC, DIM], mdt, kind="ExternalInput")
    cc_d = nc.dram_tensor("combc", [N, EPC], f32, kind="ExternalInput")
    ct_d = nc.dram_tensor("combT", [EPC, N], mdt, kind="ExternalInput")
    id_d = nc.dram_tensor("ident", [128, 128], mdt, kind="ExternalInput")
    ones_d = nc.dram_tensor("ones", [1, N], mdt, kind="ExternalInput")
    out_d = nc.dram_tensor("out", [N, DIM], f32, kind="ExternalOutput")

    with tile.TileContext(nc) as tc, ExitStack() as ctx:
        const = ctx.enter_context(tc.tile_pool(name="const", bufs=1))
        wp = ctx.enter_context(tc.tile_pool(name="wp", bufs=wbufs))
        w2p = ctx.enter_context(tc.tile_pool(name="w2p", bufs=wbufs))
        b1p = ctx.enter_context(tc.tile_pool(name="b1p", bufs=2))
        hgp = ctx.enter_context(tc.tile_pool(name="hgp", bufs=2))
        hqp = ctx.enter_context(tc.tile_pool(name="hqp", bufs=2))
        hTsp = ctx.enter_context(tc.tile_pool(name="hTsp", bufs=2))
        outp = ctx.enter_context(tc.tile_pool(name="outp", bufs=1))
        hps = ctx.enter_context(tc.tile_pool(name="hps", bufs=2, space="PSUM"))
        hTps = ctx.enter_context(tc.tile_pool(name="hTps", bufs=1, space="PSUM"))
        yps = ctx.enter_context(tc.tile_pool(name="yps", bufs=1, space="PSUM"))

        xT_sb = const.tile([128, KD * N], mdt)
        nc.sync.dma_start(
            xT_sb[:].rearrange("p (k t) -> p k t", k=KD),
            xT_d[:, :].rearrange("(k p) t -> p k t", p=128),
        )
        id_sb = const.tile([128, 128], mdt)
        nc.sync.dma_start(id_sb[:], id_d[:, :])
        cc_sb = const.tile([N, EPC], f32)
        nc.sync.dma_start(cc_sb[:], cc_d[:, :])
        ct_sb = const.tile([EPC, N], mdt)
        nc.sync.dma_start(ct_sb[:], ct_d[:, :])
        b2_sb = const.tile([EPC, DIM], mdt)
        nc.sync.dma_start(b2_sb[:], b2_d[:, :])
        ones_sb = const.tile([1, N], mdt)
        nc.sync.dma_start(ones_sb[:], ones_d[:, :])

        y_ps = yps.tile([N, DIM], f32)

        def emit_experts():
            for g in range(n_grp):
                w_t = wp.tile([128, g1w], mdt)
                csz = g1w // wsplit
                for ci in range(wsplit):
                    eng = getattr(nc, rings[ci % len(rings)])
                    eng.dma_start(w_t[:, ci * csz : (ci + 1) * csz],
                                  w1_d[g][:, ci * csz : (ci + 1) * csz])
                w2_t = w2p.tile([128, g2w], w2dt)
                c2sz = g2w // w2split
                for ci in range(w2split):
                    eng = getattr(nc, rings[(wsplit + ci) % len(rings)])
                    eng.dma_start(w2_t[:, ci * c2sz : (ci + 1) * c2sz],
                                  w2_d[g][:, ci * c2sz : (ci + 1) * c2sz])
                b1_t = b1p.tile([1, group * H], mdt)
                getattr(nc, b1eng).dma_start(
                    b1_t[:].rearrange("o (i h) -> o i h", i=group),
                    b1_d[group * g : group * (g + 1), :].rearrange(
                        "(o i) h -> o i h", o=1),
                )
                for i in range(group):
                    e = group * g + i
                    h_ps = hps.tile([N, H], f32)
                    for s in range(NSEG):
                        seg = slice(s * 512, (s + 1) * 512)
                        for k in range(KD):
                            nc.tensor.matmul(
                                h_ps[:, seg],
                                lhsT=xT_sb[:, k * N : (k + 1) * N],
                                rhs=w_t[:, i * W1B + k * H + s * 512 :
                                        i * W1B + k * H + s * 512 + 512],
                                start=(k == 0), stop=False,
                            )
                        nc.tensor.matmul(
                            h_ps[:, seg],
                            lhsT=ones_sb[:],
                            rhs=b1_t[:, i * H + s * 512 : i * H + (s + 1) * 512],
                            start=False, stop=True,
                        )

                    hg = hgp.tile([N, H], f32)
                    nc.scalar.activation(hg[:], h_ps[:], GELU)
                    # fold the top-k combine weight in per-partition; the DVE
                    # write also downcasts fp32 -> mdt for the transpose
                    hq = hqp.tile([N, H], mdt)
                    nc.vector.tensor_scalar_mul(hq[:], hg[:], cc_sb[:, e : e + 1])

                    hT_ps = hTps.tile([128, H], mdt)
                    for j in range(KH):
                        nc.tensor.transpose(
                            hT_ps[:, j * 128 : (j + 1) * 128],
                            hq[:, j * 128 : (j + 1) * 128],
                            id_sb[:],
                        )
                    hT_sb = hTsp.tile([128, H], mdt)
                    nc.vector.tensor_copy(hT_sb[:], hT_ps[:])

                    w2off = i * W2B
                    for j in range(KH):
                        nc.tensor.matmul(
                            y_ps[:],
                            lhsT=hT_sb[:, j * 128 : (j + 1) * 128],
                            rhs=w2_t[:, w2off + j * DIM : w2off + (j + 1) * DIM],
                            start=(e == 0 and j == 0), stop=False,
                        )

        if repeat > 1:
            # timing-only variant: re-run the whole expert sweep on-device
            # to amortize host/tunnel dispatch overhead. hint_engines arms
            # back-edge branch prefetch for the >256-inst PE/sync bodies so
            # the loop edge costs ~0.3us instead of a ~4us IRAM refetch.
            hint = (mybir.EngineType.PE, mybir.EngineType.SP)
            with tc.For_i(0, repeat, 1, hint_engines=hint):
                emit_experts()
        else:
            emit_experts()

        nc.tensor.matmul(
            y_ps[:], lhsT=ct_sb[:], rhs=b2_sb[:],
            start=False, stop=True,
        )
        o_sb = outp.tile([N, DIM], f32)
        nc.vector.tensor_copy(o_sb[:], y_ps[:])
        nc.sync.dma_start(out_d[:, :], o_sb[:])

    nc.compile()
    return nc


def get_program(mm_dtype_name=MM_DTYPE, act="Gelu", n_pairs=GP, repeat=1,
                wsplit=4, rings=("sync",), group=2, wbufs=2,
                b1eng="gpsimd", w2_dtype_name=W2_DTYPE, w2split=2):
    key = (mm_dtype_name, act, n_pairs, repeat, wsplit, tuple(rings), group,
           wbufs, b1eng, w2_dtype_name, w2split)
    if key not in _prog_cache:
        _prog_cache[key] = _build_program(mm_dtype_name, act, n_pairs, repeat,
                                          wsplit, rings, group, wbufs, b1eng,
                                          w2_dtype_name, w2split)
    return _prog_cache[key]


def _softmax(v, axis=-1):
    m = np.max(v, axis=axis, keepdims=True)
    ex = np.exp(v - m)
    return ex / np.sum(ex, axis=axis, keepdims=True)


def host_routing(x, router_w, router_b):
    """Replicates the reference routing in fp32 numpy: softmax over all
    experts, take top-K probs, renormalize those with another softmax."""
    xt = np.asarray(x, np.float32).reshape(N, DIM)
    logits = xt @ np.asarray(router_w, np.float32) + np.asarray(router_b, np.float32)
    probs = _softmax(logits, axis=-1)
    idx = np.argpartition(probs, E - K, axis=-1)[:, E - K:]          # top-K set
    vals = np.take_along_axis(probs, idx, axis=-1)
    w = _softmax(vals, axis=-1)
    comb = np.zeros((N, E), np.float32)
    np.put_along_axis(comb, idx, w.astype(np.float32), axis=-1)
    return comb


def pack_w1(w1c, group=2):
    """[32,512,1024] -> [32/group, 128, group*4096]: per expert group,
    per partition, w1(i,k,h) contiguous."""
    ng = EPC // group
    return np.ascontiguousarray(
        w1c.reshape(ng, group, KD_, 128, H).transpose(0, 3, 1, 2, 4)
        .reshape(ng, 128, group * W1B))


def pack_w2(w2c, group=2):
    """[32,1024,512] -> [32/group, 128, group*4096]: per expert group,
    per partition, w2(i,k,d) contiguous."""
    ng = EPC // group
    return np.ascontiguousarray(
        w2c.reshape(ng, group, KH_, 128, DIM).transpose(0, 3, 1, 2, 4)
        .reshape(ng, 128, group * W2B))


KD_ = DIM // 128
KH_ = H // 128


_NP_DT = {"float32": np.float32, "float32r": np.float32}


def _np_dt(name):
    import ml_dtypes

    return _NP_DT.get(name) or getattr(ml_dtypes, {
        "bfloat16": "bfloat16", "float16": "float16",
        "float8e3": "float8_e3m4", "float8e4": "float8_e4m3",
        "float8e5": "float8_e5m2"}[name])


def make_in_maps(x, w1, b1, w2, b2, router_w, router_b, group=2):
    hdt = _np_dt(MM_DTYPE)
    w2dt = _np_dt(W2_DTYPE) if W2_DTYPE else hdt
    w2scale = W2_SCALE if W2_DTYPE else 1.0
    x = np.ascontiguousarray(np.asarray(x, np.float32))
    w1 = np.asarray(w1, hdt)
    b1 = np.asarray(b1, hdt)
    # w2 scaled up into the fp8 normal range; compensated via combc below
    w2 = np.asarray(np.asarray(w2, np.float32) * w2scale, w2dt)
    b2 = np.asarray(b2, hdt)
    comb = host_routing(x, router_w, router_b)
    xT = np.ascontiguousarray(x.reshape(N, DIM).T.astype(hdt))
    ident = np.eye(128, dtype=hdt)
    in_maps = []
    for c in range(N_CORES):
        sl = slice(c * EPC, (c + 1) * EPC)
        cl = np.ascontiguousarray(comb[:, sl])
        in_maps.append({
            "xT": xT,
            "wpk1": pack_w1(w1[sl], group),
            "wpk2": pack_w2(w2[sl], group),
            "b1s": np.ascontiguousarray(b1[sl]),
            "b2s": np.ascontiguousarray(b2[sl]),
            "combc": np.ascontiguousarray(cl / w2scale),
            "combT": np.ascontiguousarray(cl.T.astype(hdt)),
            "ident": ident,
            "ones": np.ones((1, N), hdt),
        })
    return in_maps


def kernel(x, w1, b1, w2, b2, router_w, router_b):
    from concourse.bass_utils import run_bass_kernel_spmd

    nc = get_program()
    in_maps = make_in_maps(x, w1, b1, w2, b2, router_w, router_b)
    res = run_bass_kernel_spmd(nc, in_maps, list(range(N_CORES)))
    out = np.zeros((N, DIM), np.float32)
    for r in res.results:
        out += r["out"]
    return out.reshape(B, T, DIM).astype(np.float32)



# revision 25
# speedup vs baseline: 2.1097x; 1.0673x over previous
"""Expert-parallel MoE kernel for Trainium2 (8 NeuronCores, Bass/Tile).

Sharding: expert dim E=256 split 32-per-core across 8 cores; router is
evaluated on the host (128x256 — negligible) and each core receives its
local experts' weights plus the per-token combine weights for those
experts. Each core computes the combine-weighted partial output of its
32 experts; the host sums the 8 partials. No device collectives needed.

Weights are host-packed per expert PAIR into one contiguous block
[128 partitions x 16384 fp32] holding w1 (pre-tiled [i, k, h]) then w2
([i, k, d]); each pair streams as 4 perfectly-linear 2MB DMAs on the SP
HWDGE ring (b1 rides the gpsimd SWDGE ring so the weight stream never
stalls), measured ~97% of single-core HBM line rate.
Matmuls use float32r (fp32 bits, relaxed PE mode,
1 cyc/row) with x-transposed as the stationary operand so the streamed
weights are the moving operand. Per expert: h matmuls -> erf-GELU on
ScalarE (bias added via ones-row matmul) -> fold top-k combine weight in
with a per-partition VectorE scale -> PE transpose -> second matmul
accumulating all experts into one PSUM bank; + one K=32 matmul for the
b2 term; single output DMA.
"""

import numpy as np

B, T, DIM = 2, 64, 512
E, H, K = 256, 1024, 42
N = B * T                     # 128 tokens
N_CORES = 8
EPC = E // N_CORES            # 32 experts per core
GP = EPC // 2                 # 16 expert pairs per core

# bf16 datapath: halves the HBM weight stream vs fp32 (the kernel is
# memory-bound), PE matmuls run 1 cyc/row. Measured end-to-end rel err
# ~3e-3 vs the fp32 reference (budget 2e-2). Flip back to "float32r" if
# accuracy demands.
MM_DTYPE = "bfloat16"
# w2 streams in fp8 e3m4 (4 mantissa bits), scaled by W2_SCALE so the
# values sit in e3m4's normal range (max 15.5); the 1/W2_SCALE
# compensation is folded into the combine weights on the host, so the
# device math is unchanged. PE mixed-dtype matmul (bf16 stationary x
# fp8e3 moving) verified exact on HW. Emulated end-to-end rel err
# 1.2e-2 (budget 2e-2). Set to None to keep w2 in MM_DTYPE.
W2_DTYPE = "float8e3"
W2_SCALE = 256.0

W1B = 4 * H                   # fp32 elements of one expert's w1 per partition
W2B = 8 * DIM                 # fp32 elements of one expert's w2 per partition
PAIRW = 2 * (W1B + W2B)       # 16384 elements per partition per pair

_prog_cache = {}


def _build_program(mm_dtype_name, act="Gelu", n_pairs=GP, repeat=1,
                   wsplit=4, rings=("sync",), group=2, wbufs=2,
                   b1eng="gpsimd", w2_dtype_name=None, w2split=2,
                   variant="wstat"):
    from contextlib import ExitStack

    import concourse.bacc as bacc
    import concourse.bass as bass
    import concourse.mybir as mybir
    import concourse.tile as tile

    f32 = mybir.dt.float32
    # Matmul operands are declared in the matmul dtype end-to-end (the BIR
    # verifier requires matching producer/consumer dtypes). For bfloat16
    # the host passes ml_dtypes.bfloat16 arrays.
    mdt = getattr(mybir.dt, mm_dtype_name)
    w2dt = getattr(mybir.dt, w2_dtype_name) if w2_dtype_name else mdt
    GELU = getattr(mybir.ActivationFunctionType, act)

    KD = DIM // 128          # 4 contraction slices for x @ w1
    KH = H // 128            # 8 contraction slices for h @ w2
    NSEG = H // 512          # 2 PSUM halves for h

    nc = bacc.Bacc("TRN2", target_bir_lowering=False, debug=False,
                   num_devices=N_CORES)

    xT_d = nc.dram_tensor("xT", [DIM, N], mdt, kind="ExternalInput")
    n_grp = EPC // group
    g1w = group * W1B
    g2w = group * W2B
    w1_d = nc.dram_tensor("wpk1", [n_grp, 128, g1w], mdt, kind="ExternalInput")
    w2_d = nc.dram_tensor("wpk2", [n_grp, 128, g2w], w2dt, kind="ExternalInput")
    b2_d = nc.dram_tensor("b2s", [EPC, DIM], mdt, kind="ExternalInput")
    ct_d = nc.dram_tensor("combT", [EPC, N], mdt, kind="ExternalInput")
    # b1 pre-transposed so it lands per-partition for the post-transpose
    # gelu bias: b1T[p, e*KH + c] = b1[e, c*128 + p]
    b1T_d = nc.dram_tensor("b1T", [128, EPC * KH], f32, kind="ExternalInput")
    # combine weights (already divided by W2_SCALE) broadcast along
    # partitions: ccb[p, e*N + t] = comb[t, e] / W2_SCALE
    ccb_d = nc.dram_tensor("ccb", [128, EPC * N], mdt, kind="ExternalInput")
    out_d = nc.dram_tensor("out", [N, DIM], f32, kind="ExternalOutput")

    with tile.TileContext(nc) as tc, ExitStack() as ctx:
        const = ctx.enter_context(tc.tile_pool(name="const", bufs=1))
        wp = ctx.enter_context(tc.tile_pool(name="wp", bufs=wbufs))
        w2p = ctx.enter_context(tc.tile_pool(name="w2p", bufs=wbufs))
        hTgp = ctx.enter_context(tc.tile_pool(name="hTgp", bufs=2))
        outp = ctx.enter_context(tc.tile_pool(name="outp", bufs=1))
        hTps = ctx.enter_context(tc.tile_pool(name="hTps", bufs=2, space="PSUM"))
        yps = ctx.enter_context(tc.tile_pool(name="yps", bufs=1, space="PSUM"))

        xT_sb = const.tile([128, KD * N], mdt)
        nc.sync.dma_start(
            xT_sb[:].rearrange("p (k t) -> p k t", k=KD),
            xT_d[:, :].rearrange("(k p) t -> p k t", p=128),
        )
        ct_sb = const.tile([EPC, N], mdt)
        nc.sync.dma_start(ct_sb[:], ct_d[:, :])
        b2_sb = const.tile([EPC, DIM], mdt)
        nc.sync.dma_start(b2_sb[:], b2_d[:, :])
        b1T_sb = const.tile([128, EPC * KH], f32)
        getattr(nc, b1eng).dma_start(b1T_sb[:], b1T_d[:, :])
        ccb_sb = const.tile([128, EPC * N], mdt)
        getattr(nc, b1eng).dma_start(ccb_sb[:], ccb_d[:, :])

        y_ps = yps.tile([N, DIM], f32)

        def emit_experts():
            for g in range(n_grp):
                w_t = wp.tile([128, g1w], mdt)
                csz = g1w // wsplit
                for ci in range(wsplit):
                    eng = getattr(nc, rings[ci % len(rings)])
                    eng.dma_start(w_t[:, ci * csz : (ci + 1) * csz],
                                  w1_d[g][:, ci * csz : (ci + 1) * csz])
                w2_t = w2p.tile([128, g2w], w2dt)
                c2sz = g2w // w2split
                for ci in range(w2split):
                    eng = getattr(nc, rings[(wsplit + ci) % len(rings)])
                    eng.dma_start(w2_t[:, ci * c2sz : (ci + 1) * c2sz],
                                  w2_d[g][:, ci * c2sz : (ci + 1) * c2sz])
                for i in range(group):
                    e = group * g + i
                    # hT = (x @ w1)^T computed directly: w1 128x128 chunks
                    # stationary, xT moving. No transpose, no bias matmul.
                    hT_ps = hTps.tile([128, H], f32)
                    for hc in range(KH):
                        for k in range(KD):
                            nc.tensor.matmul(
                                hT_ps[:, hc * 128 : (hc + 1) * 128],
                                lhsT=w_t[:, i * W1B + k * H + hc * 128 :
                                         i * W1B + k * H + hc * 128 + 128],
                                rhs=xT_sb[:, k * N : (k + 1) * N],
                                start=(k == 0), stop=(k == KD - 1),
                            )
                    # b1 is per-(chunk, partition) in this layout. One DVE add
                    # over the whole tile with a stride-0-broadcast bias AP
                    # (b1T [128, KH, 1] -> [128, KH, 128]), then one big
                    # ScalarE gelu -- small per-chunk ops pay ~200ns/instr.
                    h3 = hT_ps[:].rearrange("p (c t) -> p c t", c=KH)
                    b3 = b1T_sb[:, e * KH : (e + 1) * KH].rearrange(
                        "p (c t) -> p c t", t=1)
                    h3b, b3b = bass.broadcast_tensor_aps(h3, b3)
                    nc.vector.tensor_add(h3b, h3b, b3b)
                    hTg = hTgp.tile([128, H], mdt)
                    nc.scalar.activation(hTg[:], hT_ps[:], GELU)
                    # fold the combine weight (and the 1/W2_SCALE
                    # compensation) in: per-token = per-free-column here,
                    # same [128, N] block for every chunk (stride-0 c dim)
                    g3 = hTg[:].rearrange("p (c t) -> p c t", c=KH)
                    c3 = ccb_sb[:, e * N : (e + 1) * N].rearrange(
                        "p (c t) -> p c t", c=1)
                    g3b, c3b = bass.broadcast_tensor_aps(g3, c3)
                    nc.vector.tensor_mul(g3b, g3b, c3b)
                    w2off = i * W2B
                    for j in range(KH):
                        nc.tensor.matmul(
                            y_ps[:],
                            lhsT=hTg[:, j * 128 : (j + 1) * 128],
                            rhs=w2_t[:, w2off + j * DIM : w2off + (j + 1) * DIM],
                            start=(e == 0 and j == 0), stop=False,
                        )

        if repeat > 1:
            # timing-only variant: re-run the whole expert sweep on-device
            # to amortize host/tunnel dispatch overhead. hint_engines arms
            # back-edge branch prefetch for the >256-inst PE/sync bodies so
            # the loop edge costs ~0.3us instead of a ~4us IRAM refetch.
            hint = (mybir.EngineType.PE, mybir.EngineType.SP)
            with tc.For_i(0, repeat, 1, hint_engines=hint):
                emit_experts()
        else:
            emit_experts()

        nc.tensor.matmul(
            y_ps[:], lhsT=ct_sb[:], rhs=b2_sb[:],
            start=False, stop=True,
        )
        o_sb = outp.tile([N, DIM], f32)
        nc.vector.tensor_copy(o_sb[:], y_ps[:])
        nc.sync.dma_start(out_d[:, :], o_sb[:])

    nc.compile()
    return nc


def get_program(mm_dtype_name=MM_DTYPE, act="Gelu", n_pairs=GP, repeat=1,
                wsplit=4, rings=("sync",), group=2, wbufs=3,
                b1eng="gpsimd", w2_dtype_name=W2_DTYPE, w2split=2,
                variant="wstat"):
    key = (mm_dtype_name, act, n_pairs, repeat, wsplit, tuple(rings), group,
           wbufs, b1eng, w2_dtype_name, w2split, variant)
    if key not in _prog_cache:
        _prog_cache[key] = _build_program(mm_dtype_name, act, n_pairs, repeat,
                                          wsplit, rings, group, wbufs, b1eng,
                                          w2_dtype_name, w2split, variant)
    return _prog_cache[key]


def _softmax(v, axis=-1):
    m = np.max(v, axis=axis, keepdims=True)
    ex = np.exp(v - m)
    return ex / np.sum(ex, axis=axis, keepdims=True)


def host_routing(x, router_w, router_b):
    """Replicates the reference routing in fp32 numpy: softmax over all
    experts, take top-K probs, renormalize those with another softmax."""
    xt = np.asarray(x, np.float32).reshape(N, DIM)
    logits = xt @ np.asarray(router_w, np.float32) + np.asarray(router_b, np.float32)
    probs = _softmax(logits, axis=-1)
    idx = np.argpartition(probs, E - K, axis=-1)[:, E - K:]          # top-K set
    vals = np.take_along_axis(probs, idx, axis=-1)
    w = _softmax(vals, axis=-1)
    comb = np.zeros((N, E), np.float32)
    np.put_along_axis(comb, idx, w.astype(np.float32), axis=-1)
    return comb


def pack_w1(w1c, group=2):
    """[32,512,1024] -> [32/group, 128, group*4096]: per expert group,
    per partition, w1(i,k,h) contiguous."""
    ng = EPC // group
    return np.ascontiguousarray(
        w1c.reshape(ng, group, KD_, 128, H).transpose(0, 3, 1, 2, 4)
        .reshape(ng, 128, group * W1B))


def pack_w2(w2c, group=2):
    """[32,1024,512] -> [32/group, 128, group*4096]: per expert group,
    per partition, w2(i,k,d) contiguous."""
    ng = EPC // group
    return np.ascontiguousarray(
        w2c.reshape(ng, group, KH_, 128, DIM).transpose(0, 3, 1, 2, 4)
        .reshape(ng, 128, group * W2B))


KD_ = DIM // 128
KH_ = H // 128


_NP_DT = {"float32": np.float32, "float32r": np.float32}


def _np_dt(name):
    import ml_dtypes

    return _NP_DT.get(name) or getattr(ml_dtypes, {
        "bfloat16": "bfloat16", "float16": "float16",
        "float8e3": "float8_e3m4", "float8e4": "float8_e4m3",
        "float8e5": "float8_e5m2"}[name])


def make_in_maps(x, w1, b1, w2, b2, router_w, router_b, group=2):
    hdt = _np_dt(MM_DTYPE)
    w2dt = _np_dt(W2_DTYPE) if W2_DTYPE else hdt
    w2scale = W2_SCALE if W2_DTYPE else 1.0
    x = np.ascontiguousarray(np.asarray(x, np.float32))
    w1 = np.asarray(w1, hdt)
    b1 = np.asarray(b1, hdt)
    # w2 scaled up into the fp8 normal range; compensated via combc below
    w2 = np.asarray(np.asarray(w2, np.float32) * w2scale, w2dt)
    b2 = np.asarray(b2, hdt)
    comb = host_routing(x, router_w, router_b)
    xT = np.ascontiguousarray(x.reshape(N, DIM).T.astype(hdt))
    in_maps = []
    for c in range(N_CORES):
        sl = slice(c * EPC, (c + 1) * EPC)
        cl = np.ascontiguousarray(comb[:, sl])
        b1T = (np.asarray(b1[sl], np.float32).reshape(EPC, KH_, 128)
               .transpose(2, 0, 1).reshape(128, EPC * KH_))
        ccb = np.broadcast_to((cl.T / w2scale)[None, :, :], (128, EPC, N))
        in_maps.append({
            "xT": xT,
            "wpk1": pack_w1(w1[sl], group),
            "wpk2": pack_w2(w2[sl], group),
            "b2s": np.ascontiguousarray(b2[sl]),
            "combT": np.ascontiguousarray(cl.T.astype(hdt)),
            "b1T": np.ascontiguousarray(b1T),
            "ccb": np.ascontiguousarray(ccb.reshape(128, EPC * N).astype(hdt)),
        })
    return in_maps


def kernel(x, w1, b1, w2, b2, router_w, router_b):
    from concourse.bass_utils import run_bass_kernel_spmd

    nc = get_program()
    in_maps = make_in_maps(x, w1, b1, w2, b2, router_w, router_b)
    res = run_bass_kernel_spmd(nc, in_maps, list(range(N_CORES)))
    out = np.zeros((N, DIM), np.float32)
    for r in res.results:
        out += r["out"]
    return out.reshape(B, T, DIM).astype(np.float32)

